# revision 80
# baseline (speedup 1.0000x reference)
"""Multi-head attention block (B=4, N=2048, D=1024, H=16) on 8 trn2 NeuronCores.

Sharding: core c -> (batch b = c//2, head-group g = c%2). Each core computes
attention for 8 heads of one batch plus the partial output projection over its
512 head-dims; the host sums the two partials per batch and adds b_proj.

Cost-model-driven design (matmul cost = out free-dim rows; contract dim and
out partitions are free):
  1. qkv projections use compensated-fp8 DoubleRow matmuls (0.5 cy/row,
     2x128-contraction slabs): the host splits x*16 and w*64 exactly into
     fp8e4 (hi, lo) pairs, and the device computes xh*wh + xh*wl + xl*wh,
     25% cheaper than bf16 at slightly better accuracy. The PSUM result is
     descaled by 1/1024 during the bias-add evacuation.
  2. qT/kT computed head-transposed ([dims, tokens]); v natural ([tokens,
     dims]) into vaug with a ones column per (k-tile, head) 65-col group.
  3. S^T tiles [k=128, 2 heads x 512 q] bf16 (fp8 variants exceed the 2e-2
     error gate) in a double-buffered 2-bank PSUM pool -> one exp per
     k-tile. The exp is split across ACT (native Exp) and DVE (Schraudolph:
     int16 = trunc(S*a+b) bitcast to bf16, ~2% rms, softmax cancels most of
     it): the 2-deep stab rotation couples the S stream to the exp engines,
     so alternating engines overlaps them and keeps the chain faster than
     PE. The final wave has no qkv filler left and is most exp-sensitive:
     it alternates strictly (ODD_KTS) except kt 1-2 which stay on ACT while
     DVE drains the previous block's normalize burst.
  4. PV transposed: out[q, 65] = e2[k, q]^T @ vaug[k, 65] (64 v-dims +
     denominator column); free dim 65 instead of 512. Accumulators for
     2 heads x 4 q-subtiles pack into two 1-bank PSUM tiles sharing one
     accumulation group per bank.
  5. Normalize with per-partition reciprocal scalars, hh=1 on ACT to halve
     the DVE burst; then per-128x128 DMA-engine transposes (xbar tiles;
     a wider call overwrites its 128-col output window on hw) move attn
     [q,d] -> attnT [d,q] with zero PE/DVE cost, then the output projection.
  6. Emission interleaves the S/exp stream slightly ahead of the PV stream;
     qkv work is queued as ~640ns DoubleRow chunks drained one per PV slot;
     v is computed in per-head-pair minis spread across wave 0; DMAs are
     ordered by first use with 512-token xt pieces (512B runs avoid the
     sub-512B descriptor penalty); the final 512 queries' projection
     pre-computes a pairs-0..2 partial so the drain only needs one matmul
     plus a DVE add per piece, with drain PSUM alternating between the sc
     and idle stab pools.
"""

import os
import sys

import numpy as np

try:
    import concourse.bass as bass
except ImportError:  # harness runs from a bare directory
    sys.path.insert(0, "/opt/trn_rl_repo")
    import concourse.bass as bass

import concourse.mybir as mybir
import concourse.tile as tile
from concourse.bass_utils import run_bass_kernel_spmd
from concourse.masks import make_identity

F32 = mybir.dt.float32
BF16 = mybir.dt.bfloat16
F8E4 = mybir.dt.float8e4
I16 = mybir.dt.int16
DR = mybir.MatmulPerfMode.DoubleRow
EXP = mybir.ActivationFunctionType.Exp
COPY = mybir.ActivationFunctionType.Copy
ADD = mybir.AluOpType.add
SUB = mybir.AluOpType.subtract
MULT = mybir.AluOpType.mult

B, N_FULL, D = 4, 2048, 1024
H, HD = 16, 64
NCORES = 8
GROUPS = 2          # head-groups (tensor parallel)
HL = H // GROUPS    # 8 heads per core
DL = HL * HD        # 512 local head-dims per core
PAIRS = HL // 2     # 4 head pairs
SCALE = HD ** -0.5
VG = HD + 1         # 65-col group per (k-tile, head): 64 v dims + ones col

# Compensated-fp8 qkv projection: x*16 and w*64 each split exactly into
# fp8e4 (hi, lo) pairs on the host; the projection computes
# xh*wh + xh*wl + xl*wh (dropping the ~0.1% lo*lo term) as DoubleRow
# matmuls at 0.5 cy/row, 25% cheaper than the bf16 form and slightly
# MORE accurate (≈8.5 effective mantissa bits vs bf16's 8).
QKV_DESCALE = 1.0 / 1024.0   # undo the 16*64 operand scaling at evacuation

# Schraudolph exp offload: int16 = trunc(S*SCH_A + SCH_B) bitcast to bf16
# approximates exp(S*SCALE) with ~2% rms error; the softmax normalization
# cancels most of it (measured 1.34e-2 final rel err even at 100% offload).
# Per 16-kt block, these kt slots run on DVE / Pool instead of ACT.
SCH_LOG2E = 1.4426950408889634
SCH_C = 0.0430
SCH_A = SCALE * SCH_LOG2E * 128.0
SCH_B = 128.0 * (127.0 - SCH_C) + 0.5    # +0.5: trunc -> round-half
# GPSIMD (Pool) cannot read PSUM, so only DVE shares the exp load. The
# steady state is exp-paced (the 2-buffer stab rotation couples the S
# stream to the exp engine); spreading DVE exps every 4th kt overlaps the
# two engines and drops the pace toward the PE roofline.
DVE_KTS = frozenset((1, 4, 6, 9, 12, 14))  # 6/16 of exps on DVE (waves 0-2)
# Final wave (no qkv filler left): alternate ACT/DVE so consecutive
# same-engine exps never gate the 2-deep stab rotation, but keep kt 1-2 on
# ACT while DVE drains the previous block's normalize burst.
ODD_KTS = frozenset(range(3, 16, 2))

LAST_EXEC_NS = None


def _split_multiwait_matmuls(raw: bytes) -> bytes:
    """This container's walrus allows at most one sync-wait per Matmult.

    Tile attaches up to 3. Hoist the extras onto standalone EventSemaphore
    instructions inserted immediately before the matmul on the same engine
    (identical semantics: the sequencer blocks on them in program order).
    """
    import json

    bir = json.loads(raw)
    n = [0]

    def fix_block(block):
        insts = block.get("instructions")
        if not isinstance(insts, list):
            return
        out = []
        for ins in insts:
            si = ins.get("sync_info") if isinstance(ins, dict) else None
            if (
                isinstance(ins, dict)
                and ins.get("opcode") != "EventSemaphore"
                and si
                and len(si.get("on_wait") or []) > 1
            ):
                waits = si["on_wait"]
                for w in waits[1:]:
                    n[0] += 1
                    out.append({
                        "debug": ins.get("debug", 0),
                        "engine": ins["engine"],
                        "ins": [],
                        "name": f"I-waitfix-{n[0]}",
                        "opcode": "EventSemaphore",
                        "outs": [],
                        "sync_info": {"on_update": [], "on_wait": [w]},
                    })
                si["on_wait"] = waits[:1]
            out.append(ins)
        block["instructions"] = out

    for fn in bir.get("functions", []):
        for block in fn.get("blocks", []):
            fix_block(block)
    return json.dumps(bir).encode()


def build(N=N_FULL):
    NK = N // 128   # k tiles of 128
    NQ = N // 512   # q blocks of 512
    E2_BUFS = 30
    LEAD = 2        # S-stream emission lead over the PV stream, in kt slots

    nc = bass.Bass("TRN2", target_bir_lowering=False)
    # xt packed (lo, hi) fp8 pairs of x*16; w packed (hi, lo) pairs of w*64.
    # The opposite orders make both DoubleRow cross products (xl*wh + xh*wl)
    # natural [2]-dim AP slices.
    xt = nc.dram_tensor("xt", [128, 8, 2, N], F8E4, kind="ExternalInput")
    wqk = nc.dram_tensor("wqk", [128, 4, 2, 8, 2, 128], F8E4,
                         kind="ExternalInput")
    wv = nc.dram_tensor("wv", [128, PAIRS, 8, 2, 128], F8E4,
                        kind="ExternalInput")
    bqk = nc.dram_tensor("bqk", [128, 8], F32, kind="ExternalInput")
    bv = nc.dram_tensor("bv", [128, DL], F32, kind="ExternalInput")
    wproj = nc.dram_tensor("wproj", [128, PAIRS, D], BF16, kind="ExternalInput")
    out = nc.dram_tensor("out", [N, D], BF16, kind="ExternalOutput")

    with tile.TileContext(nc) as tc:
        with (
            tc.tile_pool(name="const", bufs=1) as const_pool,
            tc.tile_pool(name="wres", bufs=1) as wres_pool,
            tc.tile_pool(name="xts", bufs=1) as xts_pool,
            tc.tile_pool(name="qk", bufs=1) as qk_pool,
            tc.tile_pool(name="vg", bufs=1) as vg_pool,
            tc.tile_pool(name="at", bufs=1) as at_pool,
            tc.tile_pool(name="ep", bufs=E2_BUFS) as e_pool,
            tc.tile_pool(name="ab", bufs=2) as ab_pool,
            tc.tile_pool(name="rp", bufs=4) as r_pool,
            tc.tile_pool(name="ob", bufs=2) as ob_pool,
            tc.tile_pool(name="psst", bufs=2, space="PSUM") as stab_pool,
            tc.tile_pool(name="pspv", bufs=1, space="PSUM") as pv_pool,
            tc.tile_pool(name="pssc", bufs=2, space="PSUM") as sc_pool,
        ):
            ident = const_pool.tile([128, 128], BF16)
            bqk_sb = const_pool.tile([128, 8], F32)
            bv_sb = const_pool.tile([128, DL], F32)
            wqk_sb = wres_pool.tile([128, 4, 2, 8, 2, 128], F8E4)
            wv_sb = wres_pool.tile([128, PAIRS, 8, 2, 128], F8E4)
            wp_sb = wres_pool.tile([128, PAIRS, D], BF16)
            # partial proj pieces (pairs 0-2) for the final 512 queries,
            # precomputed during wave 3's slack to shrink the drain
            pp_sb = wres_pool.tile([128, 4, 2, 512], BF16)
            xt_sb = xts_pool.tile([128, 8, 2, N], F8E4)
            qT = qk_pool.tile([128, PAIRS, N], BF16, tag="qT")
            kT = qk_pool.tile([128, PAIRS, N], BF16, tag="kT")
            vaug = vg_pool.tile([128, NK * HL * VG], BF16, tag="vaug")
            attnT = at_pool.tile([128, PAIRS, N], BF16, tag="attnT")

            emitted = set()

            def ensure_dma_xt(q):
                # 512-token pieces keep the per-partition runs at 512B, the
                # no-penalty DMA threshold; smaller fp8 slices pay 2x.
                key = ("xt", q)
                if key in emitted:
                    return
                emitted.add(key)
                nc.sync.dma_start(
                    xt_sb[:, :, :, q * 512:(q + 1) * 512],
                    xt[:, :, :, q * 512:(q + 1) * 512])

            def ensure_dma_xt0(h):
                ensure_dma_xt(0)

            def ensure_dma_wqk(o):
                # one DMA covers the pair's q AND k otiles (pair-major dram)
                key = ("wqk", o % 4)
                if key in emitted:
                    return
                emitted.add(key)
                nc.sync.dma_start(
                    wqk_sb[:, o % 4, :, :, :, :], wqk[:, o % 4, :, :, :, :])

            def ensure_dma_wv(p):
                key = ("wv", p)
                if key in emitted:
                    return
                emitted.add(key)
                nc.sync.dma_start(wv_sb[:, p, :, :, :], wv[:, p, :, :, :])

            # DMA priority order: the first S matmuls need wqk otiles 0
            # (q pair 0) and 4 (k pair 0) + the first xt token halves; bqk
            # is only needed at the first evacuation, after the matmuls.
            ensure_dma_wqk(0)
            ensure_dma_xt0(0)
            nc.sync.dma_start(bqk_sb[:, :], bqk[:, :])
            ensure_dma_wqk(4)
            ensure_dma_xt0(1)
            ensure_dma_wv(0)
            nc.sync.dma_start(bv_sb[:, :], bv[:, :])
            ensure_dma_xt(1)
            ensure_dma_wqk(1)
            ensure_dma_xt(2)
            ensure_dma_wv(1)
            ensure_dma_xt(3)
            ensure_dma_wqk(2)
            nc.sync.dma_start(wv_sb[:, 2:4, :, :, :], wv[:, 2:4, :, :, :])
            emitted.add(("wv", 2))
            emitted.add(("wv", 3))
            ensure_dma_wqk(3)
            nc.sync.dma_start(wp_sb[:, :, :], wproj[:, :, :])

            make_identity(nc, ident[:, :])
            # PE p-state warmup: dependency-free transposes so the tensor
            # engine reaches full clock while the first DMAs land.
            wu = sc_pool.tile([128, 512], BF16, tag="sc", name="wu")
            for _ in range(60):
                nc.tensor.matmul(
                    wu[:, 0:128], lhsT=ident[:, :], rhs=ident[:, :],
                    is_transpose=True, skip_group_check=True,
                )

            # ones column (PV denominator) for every (k-tile, head) group
            ones_view = vaug[:, :].rearrange(
                "p (g c) -> p g c", c=VG)[:, :, HD:HD + 1]
            nc.vector.tensor_scalar(
                out=ones_view,
                in0=bqk_sb[:, None, 0:1].broadcast_to([128, NK * HL, 1]),
                scalar1=0.0, scalar2=1.0, op0=MULT, op1=ADD,
            )

            # The qkv projection work is queued as ~850ns half-group chunks
            # and drained one chunk per S-slot AFTER the exp, so a chunk
            # fills the PE's stab-rotation wait instead of delaying an exp
            # (the 2-deep stab chain starves ACT whenever >1us of foreign PE
            # work lands between two S matmuls).
            filler = []
            chunks_left = {}

            def push_qk(o, ti):
                """q (o<4) / k (o>=4) projection group: 128 dims x 512 toks."""
                key = ("qk", o, ti)
                if key in chunks_left:
                    return
                chunks_left[key] = 2
                st = {}

                def half_ic(lo):
                    if lo == 0:
                        ensure_dma_wqk(o)
                        ensure_dma_xt(ti)
                        st["qp"] = sc_pool.tile(
                            [128, 512], F32, tag="sc", name="qp")
                    qp = st["qp"]
                    t0, t1 = ti * 512, (ti + 1) * 512
                    for icp in (lo, lo + 2):
                        nc.tensor.matmul(
                            qp[:, :],
                            lhsT=wqk_sb[:, o % 4, o // 4, icp:icp + 2, 0, :],
                            rhs=xt_sb[:, icp:icp + 2, 1, t0:t1],
                            start=(icp == 0), stop=False, perf_mode=DR,
                        )
                        for ic in (icp, icp + 1):
                            nc.tensor.matmul(
                                qp[:, :],
                                lhsT=wqk_sb[:, o % 4, o // 4, ic, :, :],
                                rhs=xt_sb[:, ic, :, t0:t1],
                                start=False, stop=(ic == 7), perf_mode=DR,
                            )
                    if lo == 4:
                        dst = qT if o < 4 else kT
                        nc.vector.tensor_scalar(
                            out=dst[:, o % 4, t0:t1], in0=qp[:, :],
                            scalar1=QKV_DESCALE, scalar2=bqk_sb[:, o:o + 1],
                            op0=MULT, op1=ADD,
                        )

                def half_tok(h):
                    # ti==0: split by token halves so each chunk only needs
                    # one 256-token xt DMA -- the first S/exp fires ~5us
                    # earlier during the cold start
                    if h == 0:
                        ensure_dma_wqk(o)
                        ensure_dma_xt0(0)
                        st["qp"] = sc_pool.tile(
                            [128, 512], F32, tag="sc", name="qp")
                    else:
                        ensure_dma_xt0(1)
                    qp = st["qp"]
                    t0, t1 = h * 256, (h + 1) * 256
                    for icp in (0, 2, 4, 6):
                        nc.tensor.matmul(
                            qp[:, t0:t1],
                            lhsT=wqk_sb[:, o % 4, o // 4, icp:icp + 2, 0, :],
                            rhs=xt_sb[:, icp:icp + 2, 1, t0:t1],
                            start=(h == 0 and icp == 0), stop=False,
                            perf_mode=DR, skip_group_check=True,
                        )
                        for ic in (icp, icp + 1):
                            nc.tensor.matmul(
                                qp[:, t0:t1],
                                lhsT=wqk_sb[:, o % 4, o // 4, ic, :, :],
                                rhs=xt_sb[:, ic, :, t0:t1],
                                start=False, stop=(h == 1 and ic == 7),
                                perf_mode=DR, skip_group_check=True,
                            )
                    dst = qT if o < 4 else kT
                    nc.vector.tensor_scalar(
                        out=dst[:, o % 4, t0:t1], in0=qp[:, t0:t1],
                        scalar1=QKV_DESCALE, scalar2=bqk_sb[:, o:o + 1],
                        op0=MULT, op1=ADD,
                    )

                if ti == 0:
                    filler.append((key, lambda: half_tok(0)))
                    filler.append((key, lambda: half_tok(1)))
                else:
                    filler.append((key, lambda: half_ic(0)))
                    filler.append((key, lambda: half_ic(4)))

            def push_v(s, p):
                """v projection mini for (token tile s, head pair p): only
                the pair's 2 heads (128 dims), so the v work spreads across
                all four wave-0 blocks instead of piling into the first."""
                key = ("v", s, p)
                if key in chunks_left:
                    return
                chunks_left[key] = 1

                def mini():
                    if s < 4:
                        ensure_dma_xt0(s // 2)
                    else:
                        ensure_dma_xt(s // 4)
                    ensure_dma_wv(p)
                    vp = sc_pool.tile([128, 128], F32, tag="sc", name="vp")
                    k0, k1 = s * 128, (s + 1) * 128
                    for icp in (0, 2, 4, 6):
                        nc.tensor.matmul(
                            vp[:, :],
                            lhsT=xt_sb[:, icp:icp + 2, 1, k0:k1],
                            rhs=wv_sb[:, p, icp:icp + 2, 0, :],
                            start=(icp == 0), stop=False, perf_mode=DR,
                        )
                        for ic in (icp, icp + 1):
                            nc.tensor.matmul(
                                vp[:, :],
                                lhsT=xt_sb[:, ic, :, k0:k1],
                                rhs=wv_sb[:, p, ic, :, :],
                                start=False, stop=(ic == 7), perf_mode=DR,
                            )
                    base = s * HL * VG + 2 * p * VG
                    nc.vector.scalar_tensor_tensor(
                        out=vaug[:, base:base + 2 * VG]
                        .rearrange("q (h c) -> q h c", c=VG)[:, :, 0:HD],
                        in0=vp[:, :].rearrange("q (h d) -> q h d", h=2),
                        scalar=QKV_DESCALE,
                        in1=bv_sb[:, 2 * p * HD:(2 * p + 2) * HD]
                        .rearrange("q (h d) -> q h d", h=2),
                        op0=MULT, op1=ADD,
                    )

                filler.append((key, mini))

            def push_partial(qs, e):
                key = ("pp", qs, e)
                if key in chunks_left:
                    return
                chunks_left[key] = 1

                def chunk():
                    op_ = sc_pool.tile([128, 512], F32, tag="sc", name="ppp")
                    for p_ in range(3):
                        nc.tensor.matmul(
                            op_[:, :],
                            lhsT=attnT[:, p_, (NQ - 1) * 512 + qs * 128:
                                       (NQ - 1) * 512 + (qs + 1) * 128],
                            rhs=wp_sb[:, p_, e * 512:(e + 1) * 512],
                            start=(p_ == 0),
                            stop=(p_ == 2),
                        )
                    nc.vector.tensor_copy(pp_sb[:, qs, e, :], op_[:, :])

                filler.append((key, chunk))

            def pop1():
                if filler:
                    key, fn = filler.pop(0)
                    fn()
                    chunks_left[key] -= 1

            def flush(key):
                while chunks_left.get(key, 0) > 0:
                    pop1()

            blocks = [(qn, p) for qn in range(NQ) for p in range(PAIRS)]
            e2_map = {}

            def s_stream():
                for bi, (qn, p) in enumerate(blocks):
                    push_qk(p, qn)
                    for kt in range(NK):
                        if kt % 4 == 2 and kt < 12:
                            push_qk(4 + p, kt // 4 + 1)
                        if bi + 1 < len(blocks) and kt in (4, 6, 8, 10, 12):
                            qn2, p2 = blocks[bi + 1]
                            if kt == 4:
                                push_qk(p2, qn2)
                            else:
                                push_qk(4 + p2, (kt - 6) // 2)
                        flush(("qk", p, qn))
                        flush(("qk", 4 + p, kt // 4))
                        stab = stab_pool.tile(
                            [128, 1024], F32, tag="st", name="stab")
                        for hh in (0, 1):
                            nc.tensor.matmul(
                                stab[:, hh * 512:(hh + 1) * 512],
                                lhsT=kT[hh * 64:hh * 64 + 64, p,
                                        kt * 128:(kt + 1) * 128],
                                rhs=qT[hh * 64:hh * 64 + 64, p,
                                       qn * 512:(qn + 1) * 512],
                                start=True, stop=True,
                                skip_group_check=True,
                            )
                        e2 = e_pool.tile([128, 1024], BF16, tag="e", name="e2")
                        # Final wave: the qkv filler is exhausted, so PE work
                        # per slot drops below one ACT exp; strictly alternate
                        # engines there so consecutive same-engine exps never
                        # gate the 2-deep stab rotation.
                        dve_kts = DVE_KTS if qn < NQ - 1 else ODD_KTS
                        if bi > 0 and kt in dve_kts:
                            nc.vector.tensor_scalar(
                                out=e2[:, :].bitcast(I16), in0=stab[:, :],
                                scalar1=SCH_A, scalar2=SCH_B,
                                op0=MULT, op1=ADD)
                        else:
                            nc.scalar.activation(e2[:, :], stab[:, :], EXP,
                                                 scale=SCALE)
                        e2_map[(bi, kt)] = e2
                        s_cnt[0] += 1
                        yield

            def emit_proj_piece(qn, s, e):
                op_ = sc_pool.tile([128, 512], F32, tag="sc", name="op")
                for p_ in range(PAIRS):
                    nc.tensor.matmul(
                        op_[:, :],
                        lhsT=attnT[:, p_, qn * 512 + s * 128:
                                   qn * 512 + (s + 1) * 128],
                        rhs=wp_sb[:, p_, e * 512:(e + 1) * 512],
                        start=(p_ == 0),
                        stop=(p_ == PAIRS - 1),
                    )
                ob = ob_pool.tile([128, 512], BF16, tag="ob")
                # DVE evacuation: an ACT copy here would queue between exps
                # and stretch the stab chain's ready interval
                nc.vector.tensor_copy(ob[:, :], op_[:, :])
                nc.sync.dma_start(
                    out[qn * 512 + s * 128:qn * 512 + (s + 1) * 128,
                        e * 512:(e + 1) * 512], ob[:, :])

            proj_queue = []
            pv_pos = [0]
            s_cnt = [0]
            pv_cnt = [0]

            def pv_stream():
                for bi, (qn, p) in enumerate(blocks):
                    pv_pos[0] = bi
                    pvA = pv_pool.tile([128, 4 * VG], F32, tag="pvA",
                                       name="pvA")
                    pvB = pv_pool.tile([128, 4 * VG], F32, tag="pvB",
                                       name="pvB")
                    def pv_half(hh, pv, kt):
                        # One accumulation group per PSUM bank: start marks
                        # the whole 2KB zero region pending, so only the
                        # tile's first matmul may set it.
                        e2 = e2_map[(bi, kt)]
                        vo = (kt * HL + 2 * p + hh) * VG
                        for qs in range(4):
                            nc.tensor.matmul(
                                pv[:, qs * VG:(qs + 1) * VG],
                                lhsT=e2[:, hh * 512 + qs * 128:
                                        hh * 512 + (qs + 1) * 128],
                                rhs=vaug[:, vo:vo + VG],
                                start=(kt == 0 and qs == 0),
                                stop=(kt == NK - 1 and qs == 3),
                                skip_group_check=True,
                            )

                    for kt in range(NK):
                        if qn == 0:
                            if kt == 0:
                                for s in range(3):
                                    push_v(s, p)
                            if kt + 3 < NK:
                                push_v(kt + 3, p)
                            flush(("v", kt, p))
                        pv_half(0, pvA, kt)
                        pv_half(1, pvB, kt)
                        e2_map.pop((bi, kt))
                        pop1()
                        if bi == 0:
                            pop1()
                        if kt in (5, 11) and proj_queue:
                            proj_queue.pop(0)()
                        pv_cnt[0] += 1
                        yield
                    if bi == len(blocks) - 1:
                        # Drain: normalize all 4 q-subtiles (split across
                        # ACT/DVE), one DMA-engine transpose into attnT, then
                        # the 8 final proj pieces (pair-3 matmul + pp partial
                        # add), issued qs-major so each piece starts as
                        # soon as its 128x128 transpose lands.
                        rcs = {}
                        for hh, pv in ((0, pvA), (1, pvB)):
                            pvv = pv[:, :].rearrange("p (s c) -> p s c", c=VG)
                            rc = r_pool.tile([128, 4], F32, tag="rc")
                            nc.vector.reciprocal(
                                rc[:, :, None], pvv[:, :, HD:HD + 1])
                            rcs[hh] = rc
                        ab = ab_pool.tile([128, 4, 128], BF16, tag="ab")
                        for qs in range(4):
                            for hh, pv in ((0, pvA), (1, pvB)):
                                dst = ab[:, qs, hh * 64:(hh + 1) * 64]
                                src = pv[:, qs * VG:qs * VG + HD]
                                if hh == 1:
                                    nc.scalar.activation(
                                        dst, src, COPY,
                                        scale=rcs[hh][:, qs:qs + 1])
                                else:
                                    nc.vector.tensor_scalar_mul(
                                        dst, src, rcs[hh][:, qs:qs + 1])
                            nc.sync.dma_start_transpose(
                                attnT[:, p, qn * 512 + qs * 128:
                                      qn * 512 + (qs + 1) * 128],
                                ab[:, qs, :])
                        for qs in range(4):
                            for e in range(2):
                                flush(("pp", qs, e))
                                # alternate PSUM between the sc pool and the
                                # now-idle stab pool so the 2-bank sc
                                # rotation never stalls the piece matmuls
                                if (qs * 2 + e) % 2 == 0:
                                    opf = sc_pool.tile(
                                        [128, 512], F32, tag="sc", name="opf")
                                else:
                                    opf = stab_pool.tile(
                                        [128, 1024], F32, tag="st",
                                        name="opf")[:, 0:512]
                                nc.tensor.matmul(
                                    opf[:, :],
                                    lhsT=attnT[:, 3, qn * 512 + qs * 128:
                                               qn * 512 + (qs + 1) * 128],
                                    rhs=wp_sb[:, 3, e * 512:(e + 1) * 512],
                                    start=True, stop=True,
                                )
                                ob = ob_pool.tile(
                                    [128, 512], BF16, tag="ob")
                                nc.vector.tensor_tensor(
                                    out=ob[:, :], in0=opf[:, :],
                                    in1=pp_sb[:, qs, e, :], op=ADD)
                                nc.sync.dma_start(
                                    out[qn * 512 + qs * 128:
                                        qn * 512 + (qs + 1) * 128,
                                        e * 512:(e + 1) * 512], ob[:, :])
                        yield
                        continue
                    # normalize into ab [qpart, qs, d], then DMA-engine
                    # transposes into attnT -- one per 128x128 qs-subtile
                    # (the xbar overwrites its 128-col out window for every
                    # 128-col input group, so a single 512-wide call is
                    # wrong on hw).
                    ab = ab_pool.tile([128, 4, 128], BF16, tag="ab")
                    for hh, pv in ((0, pvA), (1, pvB)):
                        pvv = pv[:, :].rearrange("p (s c) -> p s c", c=VG)
                        rc = r_pool.tile([128, 4], F32, tag="rc")
                        nc.vector.reciprocal(
                            rc[:, :, None], pvv[:, :, HD:HD + 1])
                        for qs in range(4):
                            dst = ab[:, qs, hh * 64:(hh + 1) * 64]
                            src = pv[:, qs * VG:qs * VG + HD]
                            if hh == 1:
                                # ACT has slack; halving the DVE burst keeps
                                # queued DVE exps from gating the stab chain
                                nc.scalar.activation(
                                    dst, src, COPY,
                                    scale=rc[:, qs:qs + 1])
                            else:
                                nc.vector.tensor_scalar_mul(
                                    dst, src, rc[:, qs:qs + 1])
                    yield
                    for qs in range(4):
                        nc.sync.dma_start_transpose(
                            attnT[:, p, qn * 512 + qs * 128:
                                  qn * 512 + (qs + 1) * 128],
                            ab[:, qs, :])
                    yield
                    if bi == len(blocks) - 2:
                        for qs_ in range(4):
                            for e_ in range(2):
                                push_partial(qs_, e_)
                    if p == PAIRS - 1 and qn < NQ - 1:
                        for s in range(4):
                            for e in range(2):
                                proj_queue.append(
                                    lambda qn=qn, s=s, e=e:
                                    emit_proj_piece(qn, s, e))
                    yield

            sg, pg = s_stream(), pv_stream()

            def step(g):
                try:
                    next(g)
                    return True
                except StopIteration:
                    return False

            # seed block 0's projection groups and the first v minis
            push_qk(0, 0)
            push_qk(4, 0)
            for s in range(3):
                push_v(s, 0)
            for _ in range(LEAD):
                step(sg)
            s_live = p_live = True
            while s_live or p_live:
                # PV first: its operands are long ready, so the PE never
                # head-of-line blocks on a stab-rotation wait inside S.
                if p_live:
                    p_live = step(pg)
                if s_live:
                    s_live = step(sg)
                if s_live and pv_pos[0] < 1:
                    # block 0 is PE-bound: run the S/exp stream ahead so ACT
                    # banks exps (bounded by the e2 pool rotation)
                    s_live = step(sg)

            while proj_queue:
                proj_queue.pop(0)()

    _orig_to_json = nc.to_json_bytes
    nc.to_json_bytes = lambda: _split_multiwait_matmuls(_orig_to_json())
    return nc


def shard_inputs(x, w_qkv, b_qkv, w_proj, N=N_FULL):
    """Build the 8 per-core input maps from full inputs.

    x*16 and w*64 are each split exactly into fp8e4 (hi, lo) pairs for the
    DoubleRow qkv projection; x packs (lo, hi), w packs (hi, lo) so the
    device's cross products are natural slab slices.
    """
    import ml_dtypes

    bf16 = ml_dtypes.bfloat16
    f8 = ml_dtypes.float8_e4m3

    def split_f8(a, scale):
        s = np.asarray(a * scale, np.float32)
        hi = s.astype(f8)
        lo = (s - hi.astype(np.float32)).astype(f8)
        return hi, lo

    x = np.asarray(x, dtype=np.float32)
    w_qkv = np.asarray(w_qkv, dtype=np.float32)
    b_qkv = np.asarray(b_qkv, dtype=np.float32)
    w_proj = np.asarray(w_proj, dtype=np.float32)
    in_maps = []
    for c in range(NCORES):
        b, g = divmod(c, 2)
        qc = slice(g * DL, (g + 1) * DL)
        wq = w_qkv[:, 0 * D:1 * D][:, qc]
        wk = w_qkv[:, 1 * D:2 * D][:, qc]
        wv_ = w_qkv[:, 2 * D:3 * D][:, qc]
        wqk_np = np.empty((128, 4, 2, 8, 128), np.float32)
        bqk_np = np.empty((128, 8), np.float32)
        for o in range(8):
            wsrc = wq if o < 4 else wk
            bsrc = b_qkv[0:D][qc] if o < 4 else b_qkv[D:2 * D][qc]
            blk = wsrc[:, (o % 4) * 128:(o % 4 + 1) * 128].reshape(8, 128, 128)
            wqk_np[:, o % 4, o // 4] = blk.transpose(1, 0, 2)
            bqk_np[:, o] = bsrc[(o % 4) * 128:(o % 4 + 1) * 128]
        wv_np = wv_.reshape(8, 128, PAIRS, 128).transpose(1, 2, 0, 3)
        bv_np = np.broadcast_to(b_qkv[2 * D:3 * D][qc], (128, DL)).copy()
        wp_np = w_proj[g * DL:(g + 1) * DL, :].reshape(
            PAIRS, 128, D).transpose(1, 0, 2)
        xb = x[min(b, x.shape[0] - 1), :N] if x.ndim == 3 else x[:N]
        # xt[p, ic, t] = x[t, ic*128 + p]
        xt_np = xb.T.reshape(8, 128, N).transpose(1, 0, 2)
        xh, xl = split_f8(xt_np, 16.0)
        xt8 = np.stack([xl, xh], axis=2)                # [128, 8, 2, N]
        qh, ql = split_f8(wqk_np, 64.0)
        wqk8 = np.stack([qh, ql], axis=4)               # [128, 4, 2, 8, 2, 128]
        vh, vl = split_f8(wv_np, 64.0)
        wv8 = np.stack([vh, vl], axis=3)                # [128, PAIRS, 8, 2, 128]
        in_maps.append({
            "xt": np.ascontiguousarray(xt8),
            "wqk": np.ascontiguousarray(wqk8),
            "wv": np.ascontiguousarray(wv8),
            "bqk": np.ascontiguousarray(bqk_np),
            "bv": np.ascontiguousarray(bv_np),
            "wproj": np.ascontiguousarray(wp_np).astype(bf16),
        })
    return in_maps


_NC_CACHE = {}


def kernel(x, w_qkv, b_qkv, w_proj, b_proj):
    global LAST_EXEC_NS
    x = np.asarray(x, dtype=np.float32)
    b_proj = np.asarray(b_proj, dtype=np.float32)
    if N_FULL not in _NC_CACHE:
        _NC_CACHE[N_FULL] = build(N_FULL)
    nc = _NC_CACHE[N_FULL]
    in_maps = shard_inputs(x, w_qkv, b_qkv, w_proj)
    trace = os.environ.get("KERNEL_TRACE", "0") == "1"
    res = run_bass_kernel_spmd(
        nc, in_maps, core_ids=list(range(NCORES)), trace=trace,
        trace_cores=[0] if trace else None,
    )
    LAST_EXEC_NS = res.exec_time_ns
    outs = [np.asarray(r["out"], dtype=np.float32) for r in res.results]
    full = np.empty((B, N_FULL, D), np.float32)
    for b in range(B):
        full[b] = outs[2 * b] + outs[2 * b + 1]
    full += b_proj[None, None, :]
    return full

